# revision 9
# baseline (speedup 1.0000x reference)
"""Trainium2 Bass kernel for 16-head MHA (B=2, S=2048, D=1024, E=64).

Sharding: 8 cores = 2 batches x 4 head-groups. Each core computes 4 heads
(2 pairs of 2) for one batch and returns a partial output [2048, 1024]
(sum of its 4 heads' contributions after the output projection) in fp16.
Host sums the 4 partials per batch.

Per-core schedule (software-pipelined so the ACT engine, which owns the
16.8M-element exp, is saturated from ~12us):
  - K projection, then Q chunk 0, then 8 "steps" (query-chunk x pair).
  - Step s runs S^T+exp for (qc,p); AV matmuls of step s-1 and the output
    projection of step s-3 ride inside its kb loop; the V projection
    (computed token-major on the PE - no DMA transposes) rides step 0 and
    remaining Q chunks ride step 1.
  - softmax denominators via the [V|1] ones column; normalization uses
    reciprocal_approx_fast + gpsimd partition-broadcast, multiplied
    straight out of PSUM into fp16 O^T.
"""

import sys

sys.path.insert(0, "/opt/trn_rl_repo")

import numpy as np

import concourse.bass as bass
import concourse.bacc as bacc
import concourse.mybir as mybir
from concourse import tile
from concourse.tile_rust import add_dep_helper
from concourse.bass_interp import get_hw_module
from concourse.bass_utils import run_bass_kernel_spmd

F16 = mybir.dt.float16
F32 = mybir.dt.float32
BF16 = mybir.dt.bfloat16

N_CORES = 8
T = 2048          # tokens per core (one batch)
D = 1024          # model dim
E = 64            # head dim
QC = 512          # query chunk
NQ = T // QC      # 4 query chunks
KB = 128          # key block
NKB = T // KB     # 16 key blocks
ND = D // 128     # 8 contraction chunks for projections

_CACHE = {}


def _build():
    nc = bacc.Bacc("TRN2", target_bir_lowering=False, debug=False,
                   num_devices=N_CORES)

    xqT = nc.dram_tensor("xqT", [D, T], F16, kind="ExternalInput").ap()
    xkT = nc.dram_tensor("xkT", [D, T], F16, kind="ExternalInput").ap()
    xvT = nc.dram_tensor("xvT", [D, T], F16, kind="ExternalInput").ap()
    # per-pair packed weights, layout [128, 8*128]: chunk d at cols d*128
    wq = [nc.dram_tensor(f"wq{p}", [128, D], F16, kind="ExternalInput").ap()
          for p in range(2)]
    wk = [nc.dram_tensor(f"wk{p}", [128, D], F16, kind="ExternalInput").ap()
          for p in range(2)]
    # all-4-head V weights for token-major projection: chunk d at cols d*256
    wv4 = nc.dram_tensor("wv4", [128, ND * 256], F16, kind="ExternalInput").ap()
    wo = [nc.dram_tensor(f"wo{p}", [128, D], F16, kind="ExternalInput").ap()
          for p in range(2)]
    pout = nc.dram_tensor("pout", [T, D], F16, kind="ExternalOutput").ap()

    with tile.TileContext(nc) as tc:
        with (
            tc.tile_pool(name="consts", bufs=1) as consts,
            tc.tile_pool(name="persist", bufs=1) as persist,
            tc.tile_pool(name="xs", bufs=1) as xs,
            tc.tile_pool(name="at", bufs=18) as atp,
            tc.tile_pool(name="o2t", bufs=2) as o2tp,
            tc.tile_pool(name="os", bufs=3) as osp,
            tc.tile_pool(name="small", bufs=1) as smallp,
            tc.tile_pool(name="psS", bufs=2, space="PSUM") as psS,
            tc.tile_pool(name="psO", bufs=1, space="PSUM") as psO,
            tc.tile_pool(name="psX", bufs=2, space="PSUM") as psX,
        ):
            # ---- weights ----
            wq_sb = [consts.tile([128, D], F16, tag=f"wq{p}", name=f"wq_sb{p}") for p in range(2)]
            wk_sb = [consts.tile([128, D], F16, tag=f"wk{p}", name=f"wk_sb{p}") for p in range(2)]
            wo_sb = [consts.tile([128, D], F16, tag=f"wo{p}", name=f"wo_sb{p}") for p in range(2)]
            wv4_sb = consts.tile([128, ND * 256], F16, tag="wv4", name="wv4_sb")
            for p in range(2):
                nc.gpsimd.dma_start(wk_sb[p][:], wk[p][:])
            nc.gpsimd.dma_start(wv4_sb[:], wv4[:])
            for p in range(2):
                nc.gpsimd.dma_start(wq_sb[p][:], wq[p][:])
                nc.gpsimd.dma_start(wo_sb[p][:], wo[p][:])

            # ---- persistent activations ----
            qt = [[persist.tile([128, QC], F16, tag=f"qt{p}_{t}", name=f"qt{p}_{t}")
                   for t in range(NQ)] for p in range(2)]
            kt = [persist.tile([128, T], F16, tag=f"kt{p}", name=f"kt{p}") for p in range(2)]
            # token(key)-major [V | 1] per (head, key-block): [128, 65] each
            v2 = [[persist.tile([128, 65], BF16, tag=f"v2_{h}_{b}", name=f"v2_{h}_{b}")
                   for b in range(NKB)] for h in range(4)]
            ones_f32 = consts.tile([128, 1], F32, tag="ones", name="ones_f32")
            nc.vector.memset(ones_f32[:], 1.0)
            for h in range(4):
                for b in range(NKB):
                    nc.vector.tensor_copy(v2[h][b][:, 64:65], ones_f32[:])

            # ---- input tiles + staged DMA issue ----
            xk = [xs.tile([128, T], F16, tag=f"xk{d}", name=f"xk{d}") for d in range(ND)]
            xq = [xs.tile([128, T], F16, tag=f"xq{d}", name=f"xq{d}") for d in range(ND)]
            xv = [xs.tile([128, T], F16, tag=f"xv{d}", name=f"xv{d}") for d in range(ND)]
            # sync queue: the critical prefixes
            for d in range(ND):
                nc.sync.dma_start(xk[d][:, 0:T // 2], xkT[d * 128:(d + 1) * 128, 0:T // 2])
            for d in range(ND):
                nc.sync.dma_start(xq[d][:, 0:QC], xqT[d * 128:(d + 1) * 128, 0:QC])
            for d in range(ND):
                nc.sync.dma_start(xk[d][:, T // 2:T], xkT[d * 128:(d + 1) * 128, T // 2:T])
            # gpsimd queue: V and the Q remainder
            for d in range(ND):
                nc.gpsimd.dma_start(xv[d][:, 0:T // 2], xvT[d * 128:(d + 1) * 128, 0:T // 2])
            for d in range(ND):
                nc.gpsimd.dma_start(xq[d][:, QC:T], xqT[d * 128:(d + 1) * 128, QC:T])
            for d in range(ND):
                nc.gpsimd.dma_start(xv[d][:, T // 2:T], xvT[d * 128:(d + 1) * 128, T // 2:T])

            # ---- helpers ----
            def proj_chains(specs, pfx):
                """specs: list of (x, w_sb_p, dst_ap, dst_slice). Interleaves
                the psum accumulation chains (2 at a time) so consecutive PE
                matmuls hit different banks and pipeline."""
                for c0 in range(0, len(specs), 2):
                    grp = specs[c0:c0 + 2]
                    pss = [psX.tile([128, QC], F32, tag="x", name=f"ps_{pfx}_{c0 + i}")
                           for i in range(len(grp))]
                    for d in range(ND):
                        for i, (x, w_sb_p, dst, sl) in enumerate(grp):
                            nc.tensor.matmul(
                                pss[i][:], w_sb_p[:, d * 128:(d + 1) * 128],
                                x[d][:, sl], start=(d == 0), stop=(d == ND - 1))
                    for i, (x, w_sb_p, dst, sl) in enumerate(grp):
                        nc.vector.tensor_copy(dst, pss[i][:])

            # before the pipeline: only K pair0 and Q chunk 0 of pair0
            proj_chains([(xk, wk_sb[0], kt[0][:, t * QC:(t + 1) * QC],
                          slice(t * QC, (t + 1) * QC)) for t in range(NQ)], "k0")
            proj_chains([(xq, wq_sb[0], qt[0][0][:], slice(0, QC))], "q00")

            def emit_vproj(kc):
                """token-major V projection for key block kc: all 4 heads."""
                psv = psX.tile([128, 256], F32, tag="x", name=f"psv_{kc}")
                for d in range(ND):
                    nc.tensor.matmul(
                        psv[:], xv[d][:, kc * 128:(kc + 1) * 128],
                        wv4_sb[:, d * 256:(d + 1) * 256],
                        start=(d == 0), stop=(d == ND - 1))
                for h in range(4):
                    nc.vector.tensor_copy(v2[h][kc][:, 0:64],
                                          psv[:, h * 64:(h + 1) * 64])

            def emit_qproj(t):
                sl = slice(t * QC, (t + 1) * QC)
                proj_chains([(xq, wq_sb[p], qt[p][t][:], sl) for p in range(2)],
                            f"q_{t}")

            # ---- output projection ----
            ost_live = {}

            def emit_outproj_group(qc, o2t_pair, sub, oc, anchor):
                q0 = qc * QC
                if oc == 0:
                    ost_live[(qc, sub)] = osp.tile(
                        [128, D], F16, tag="os", name=f"os_{qc}_{sub}")
                ost = ost_live[(qc, sub)]
                pp = psX.tile([128, 512], F32, tag="x", name=f"pp_{qc}_{sub}_{oc}")
                for p in range(2):
                    mm = nc.tensor.matmul(
                        pp[:],
                        o2t_pair[p][:, sub * 128:(sub + 1) * 128],
                        wo_sb[p][:, oc * 512:(oc + 1) * 512],
                        start=(p == 0), stop=(p == 1))
                    if p == 0 and anchor is not None:
                        add_dep_helper(mm.ins, anchor.ins, sync=False,
                                       reason="interleave outproj after S")
                nc.vector.tensor_copy(ost[:, oc * 512:(oc + 1) * 512], pp[:])
                if oc == 1:
                    nc.sync.dma_start(
                        pout[q0 + sub * 128:q0 + (sub + 1) * 128, :],
                        ost[:])
                    del ost_live[(qc, sub)]

            # ---- 8-step attention pipeline ----
            steps = [(qc, p) for qc in range(NQ) for p in range(2)]
            prev = None           # (qc, p, po[2], at_list)
            o2t_all = {}          # (qc, p) -> o2t tile [128, QC] f16

            def emit_av(pr, kb):
                pqc, pp_, ppo, pat = pr
                for h in range(2):
                    nc.tensor.matmul(
                        ppo[h][:],
                        v2[2 * pp_ + h][kb][:],
                        pat[kb][:, h * QC:(h + 1) * QC],
                        start=(kb == 0), stop=(kb == NKB - 1))

            def emit_norm(pr):
                pqc, pp_, ppo, pat = pr
                o2t_t = o2tp.tile([128, QC], F16, tag=f"o2t{pp_}",
                                  name=f"o2t_{pqc}_{pp_}")
                for h in range(2):
                    d_sb = smallp.tile([1, QC], F32, tag=f"d{h}", name=f"d_{pqc}_{pp_}_{h}")
                    nc.vector.tensor_copy(d_sb[:], ppo[h][64:65, :])
                    r = smallp.tile([1, QC], F32, tag=f"r{h}", name=f"r_{pqc}_{pp_}_{h}")
                    nc.vector.reciprocal_approx_fast(r[:], d_sb[:])
                    rb = smallp.tile([64, QC], F32, tag=f"rb{h}", name=f"rb_{pqc}_{pp_}_{h}")
                    nc.gpsimd.partition_broadcast(rb[:], r[:])
                    nc.vector.tensor_mul(
                        o2t_t[h * 64:(h + 1) * 64, :],
                        ppo[h][0:64, :], rb[:])
                o2t_all[(pqc, pp_)] = o2t_t

            for s, (qc, p) in enumerate(steps):
                po = [psO.tile([65, QC], F32, tag=f"o{h}", name=f"po_{qc}_{p}_{h}")
                      for h in range(2)]
                at_list = []
                for kb in range(NKB):
                    k0 = kb * KB
                    ps = psS.tile([128, 2 * QC], F32, tag="s", name=f"s_{qc}_{p}_{kb}")
                    s_anchor = nc.tensor.matmul(
                        ps[:, 0:QC],
                        kt[p][0:64, k0:k0 + KB],
                        qt[p][qc][0:64, :],
                        start=True, stop=True, tile_position=(0, 0))
                    nc.tensor.matmul(
                        ps[:, QC:2 * QC],
                        kt[p][64:128, k0:k0 + KB],
                        qt[p][qc][64:128, :],
                        start=True, stop=True, tile_position=(64, 0))
                    at_t = atp.tile([128, 2 * QC], BF16, tag="at", name=f"at_{qc}_{p}_{kb}")
                    nc.scalar.activation(
                        at_t[:], ps[:], mybir.ActivationFunctionType.Exp)
                    at_list.append(at_t)

                    if s == 0:
                        emit_vproj(kb)
                        if kb == 2:   # K pair1, t0+t1
                            proj_chains(
                                [(xk, wk_sb[1], kt[1][:, t * QC:(t + 1) * QC],
                                  slice(t * QC, (t + 1) * QC)) for t in (0, 1)],
                                "k1a")
                        if kb == 8:   # K pair1, t2+t3
                            proj_chains(
                                [(xk, wk_sb[1], kt[1][:, t * QC:(t + 1) * QC],
                                  slice(t * QC, (t + 1) * QC)) for t in (2, 3)],
                                "k1b")
                        if kb == 12:  # Q pair1, chunk 0
                            proj_chains([(xq, wq_sb[1], qt[1][0][:], slice(0, QC))],
                                        "q10")
                    if s == 1 and kb in (2, 7, 12):
                        emit_qproj({2: 1, 7: 2, 12: 3}[kb])
                    if prev is not None:
                        emit_av(prev, kb)
                    # outproj of qc'=(s-3)//2 rides odd steps >= 3
                    if s >= 3 and s % 2 == 1 and kb % 2 == 1:
                        oqc = (s - 3) // 2
                        emit_outproj_group(
                            oqc,
                            [o2t_all[(oqc, 0)], o2t_all[(oqc, 1)]],
                            kb // 4, (kb // 2) % 2, s_anchor)
                if prev is not None:
                    emit_norm(prev)
                prev = (qc, p, po, at_list)

            # ---- tail: AV + norm of the last step, outproj of qc=3 ----
            for kb in range(NKB):
                emit_av(prev, kb)
            emit_norm(prev)
            for sub in range(4):
                for oc in range(2):
                    emit_outproj_group(
                        3, [o2t_all[(3, 0)], o2t_all[(3, 1)]], sub, oc, None)

    nc.compile()
    nc.m = get_hw_module(nc.m)
    return nc


def _pack_w(w_pair):
    # w_pair: [2, 1024, 64] -> [1024, 128] -> chunk-major [128, 8*128]
    w = np.concatenate([w_pair[0], w_pair[1]], axis=1)          # [1024, 128]
    return np.ascontiguousarray(
        w.reshape(ND, 128, 128).transpose(1, 0, 2).reshape(128, D))


def _pack_wv4(w_quad):
    # w_quad: [4, 1024, 64] -> [1024, 256] -> chunk-major [128, 8*256]
    w = np.concatenate([w_quad[h] for h in range(4)], axis=1)   # [1024, 256]
    return np.ascontiguousarray(
        w.reshape(ND, 128, 256).transpose(1, 0, 2).reshape(128, ND * 256))


def _pack_wo(wo_pair):
    # wo_pair: [2, 64, 1024] -> [128, 1024]
    return np.ascontiguousarray(np.concatenate([wo_pair[0], wo_pair[1]], axis=0))


def kernel(q, k, v, W_query, W_key, W_val, W_out, _trace=False):
    q = np.asarray(q, dtype=np.float32)
    k = np.asarray(k, dtype=np.float32)
    v = np.asarray(v, dtype=np.float32)
    W_query = np.asarray(W_query, dtype=np.float32)
    W_key = np.asarray(W_key, dtype=np.float32)
    W_val = np.asarray(W_val, dtype=np.float32)
    W_out = np.asarray(W_out, dtype=np.float32)

    if "nc" not in _CACHE:
        _CACHE["nc"] = _build()
    nc = _CACHE["nc"]

    norm = 1.0 / np.sqrt(E)
    xT = {}
    for b in range(2):
        xT[("q", b)] = np.ascontiguousarray(q[b].T).astype(np.float16)
        xT[("k", b)] = np.ascontiguousarray(k[b].T).astype(np.float16)
        xT[("v", b)] = np.ascontiguousarray(v[b].T).astype(np.float16)

    in_maps = []
    for c in range(N_CORES):
        b, g = c // 4, c % 4
        hs = [4 * g, 4 * g + 1, 4 * g + 2, 4 * g + 3]
        m = {
            "xqT": xT[("q", b)], "xkT": xT[("k", b)], "xvT": xT[("v", b)],
            "wv4": _pack_wv4(W_val[hs]).astype(np.float16),
        }
        for p in range(2):
            hp = hs[2 * p:2 * p + 2]
            m[f"wq{p}"] = _pack_w(W_query[hp] * norm).astype(np.float16)
            m[f"wk{p}"] = _pack_w(W_key[hp]).astype(np.float16)
            m[f"wo{p}"] = _pack_wo(W_out[hp]).astype(np.float16)
        in_maps.append(m)

    res = run_bass_kernel_spmd(nc, in_maps, list(range(N_CORES)),
                               trace=_trace)
    parts = [res.results[c]["pout"].astype(np.float32) for c in range(N_CORES)]
    out = np.stack([
        parts[0] + parts[1] + parts[2] + parts[3],
        parts[4] + parts[5] + parts[6] + parts[7],
    ])
    if _trace:
        _CACHE["last_result"] = res
    return out


# revision 17
# speedup vs baseline: 1.1160x; 1.1160x over previous
"""Trainium2 Bass kernel for 16-head MHA (B=2, S=2048, D=1024, E=64).

Sharding: 8 cores = 2 batches x 4 head-groups. Each core computes 4 heads
(2 pairs of 2) for one batch and returns a partial output [2048, 1024]
(sum of its 4 heads' contributions after the output projection) in fp16.
Host sums the 4 partials per batch.

Per-core schedule (software-pipelined so the ACT engine, which owns the
16.8M-element exp, is saturated from ~12us):
  - K projection, then Q chunk 0, then 8 "steps" (query-chunk x pair).
  - Step s runs S^T+exp for (qc,p); AV matmuls of step s-1 and the output
    projection of step s-3 ride inside its kb loop; the V projection
    (computed token-major on the PE - no DMA transposes) rides step 0 and
    remaining Q chunks ride step 1.
  - softmax denominators via the [V|1] ones column; normalization uses
    reciprocal_approx_fast + gpsimd partition-broadcast, multiplied
    straight out of PSUM into fp16 O^T.
"""

import sys

sys.path.insert(0, "/opt/trn_rl_repo")

import numpy as np

import concourse.bass as bass
import concourse.bacc as bacc
import concourse.mybir as mybir
from concourse import tile
from concourse.tile_rust import add_dep_helper
from concourse.bass_interp import get_hw_module
from concourse.bass_utils import run_bass_kernel_spmd

F16 = mybir.dt.float16
F32 = mybir.dt.float32
BF16 = mybir.dt.bfloat16

N_CORES = 8
T = 2048          # tokens per core (one batch)
D = 1024          # model dim
E = 64            # head dim
QC = 512          # query chunk
NQ = T // QC      # 4 query chunks
KB = 128          # key block
NKB = T // KB     # 16 key blocks
ND = D // 128     # 8 contraction chunks for projections

_CACHE = {}


def _build():
    nc = bacc.Bacc("TRN2", target_bir_lowering=False, debug=False,
                   num_devices=N_CORES)

    xqT = nc.dram_tensor("xqT", [D, T], F16, kind="ExternalInput").ap()
    xkT = nc.dram_tensor("xkT", [D, T], F16, kind="ExternalInput").ap()
    xvT = nc.dram_tensor("xvT", [D, T], F16, kind="ExternalInput").ap()
    # per-pair packed weights, layout [128, 8*128]: chunk d at cols d*128
    wq = [nc.dram_tensor(f"wq{p}", [128, D], F16, kind="ExternalInput").ap()
          for p in range(2)]
    wk = [nc.dram_tensor(f"wk{p}", [128, D], F16, kind="ExternalInput").ap()
          for p in range(2)]
    # all-4-head V weights for token-major projection: chunk d at cols d*256
    wv4 = nc.dram_tensor("wv4", [128, ND * 256], F16, kind="ExternalInput").ap()
    wo = [nc.dram_tensor(f"wo{p}", [128, D], F16, kind="ExternalInput").ap()
          for p in range(2)]
    pout = nc.dram_tensor("pout", [T, D], F16, kind="ExternalOutput").ap()

    with tile.TileContext(nc) as tc:
        with (
            tc.tile_pool(name="consts", bufs=1) as consts,
            tc.tile_pool(name="persist", bufs=1) as persist,
            tc.tile_pool(name="xs", bufs=1) as xs,
            tc.tile_pool(name="at", bufs=18) as atp,
            tc.tile_pool(name="o2t", bufs=2) as o2tp,
            tc.tile_pool(name="os", bufs=3) as osp,
            tc.tile_pool(name="small", bufs=1) as smallp,
            tc.tile_pool(name="psS", bufs=2, space="PSUM") as psS,
            tc.tile_pool(name="psO", bufs=1, space="PSUM") as psO,
            tc.tile_pool(name="psX", bufs=2, space="PSUM") as psX,
        ):
            # ---- weights ----
            wq_sb = [consts.tile([128, D], F16, tag=f"wq{p}", name=f"wq_sb{p}") for p in range(2)]
            wk_sb = [consts.tile([128, D], F16, tag=f"wk{p}", name=f"wk_sb{p}") for p in range(2)]
            wo_sb = [consts.tile([128, D], F16, tag=f"wo{p}", name=f"wo_sb{p}") for p in range(2)]
            wv4_sb = consts.tile([128, ND * 256], F16, tag="wv4", name="wv4_sb")
            for p in range(2):
                nc.gpsimd.dma_start(wk_sb[p][:], wk[p][:])
            nc.gpsimd.dma_start(wv4_sb[:], wv4[:])
            for p in range(2):
                nc.gpsimd.dma_start(wq_sb[p][:], wq[p][:])
                nc.gpsimd.dma_start(wo_sb[p][:], wo[p][:])

            # ---- persistent activations ----
            qt = [[persist.tile([128, QC], F16, tag=f"qt{p}_{t}", name=f"qt{p}_{t}")
                   for t in range(NQ)] for p in range(2)]
            kt = [persist.tile([128, T], F16, tag=f"kt{p}", name=f"kt{p}") for p in range(2)]
            # token(key)-major [V | 1] per (head, key-block): [128, 65] each
            v2 = [[persist.tile([128, 65], BF16, tag=f"v2_{h}_{b}", name=f"v2_{h}_{b}")
                   for b in range(NKB)] for h in range(4)]
            ones_f32 = consts.tile([128, 1], F32, tag="ones", name="ones_f32")
            nc.vector.memset(ones_f32[:], 1.0)
            for h in range(4):
                for b in range(NKB):
                    nc.vector.tensor_copy(v2[h][b][:, 64:65], ones_f32[:])

            # ---- input tiles + staged DMA issue ----
            xk = [xs.tile([128, T], F16, tag=f"xk{d}", name=f"xk{d}") for d in range(ND)]
            xq = [xs.tile([128, T], F16, tag=f"xq{d}", name=f"xq{d}") for d in range(ND)]
            xv = [xs.tile([128, T], F16, tag=f"xv{d}", name=f"xv{d}") for d in range(ND)]
            # two DMA queues; split the critical xk first half across both
            for d in range(4):
                nc.sync.dma_start(xk[d][:, 0:T // 2], xkT[d * 128:(d + 1) * 128, 0:T // 2])
            for d in range(4, ND):
                nc.gpsimd.dma_start(xk[d][:, 0:T // 2], xkT[d * 128:(d + 1) * 128, 0:T // 2])
            for d in range(ND):
                nc.sync.dma_start(xq[d][:, 0:QC], xqT[d * 128:(d + 1) * 128, 0:QC])
            for d in range(ND):
                nc.gpsimd.dma_start(xv[d][:, 0:T // 2], xvT[d * 128:(d + 1) * 128, 0:T // 2])
            for d in range(ND):
                nc.sync.dma_start(xk[d][:, T // 2:T], xkT[d * 128:(d + 1) * 128, T // 2:T])
            for d in range(ND):
                nc.sync.dma_start(xq[d][:, QC:T], xqT[d * 128:(d + 1) * 128, QC:T])
            for d in range(ND):
                nc.gpsimd.dma_start(xv[d][:, T // 2:T], xvT[d * 128:(d + 1) * 128, T // 2:T])

            # ---- helpers ----
            def proj_chains(specs, pfx):
                """specs: list of (x, w_sb_p, dst_ap, dst_slice). Interleaves
                the psum accumulation chains (2 at a time) so consecutive PE
                matmuls hit different banks and pipeline."""
                for c0 in range(0, len(specs), 2):
                    grp = specs[c0:c0 + 2]
                    pss = [psX.tile([128, QC], F32, tag="x", name=f"ps_{pfx}_{c0 + i}")
                           for i in range(len(grp))]
                    for d in range(ND):
                        for i, (x, w_sb_p, dst, sl) in enumerate(grp):
                            nc.tensor.matmul(
                                pss[i][:], w_sb_p[:, d * 128:(d + 1) * 128],
                                x[d][:, sl], start=(d == 0), stop=(d == ND - 1))
                    for i, (x, w_sb_p, dst, sl) in enumerate(grp):
                        nc.vector.tensor_copy(dst, pss[i][:])

            def kproj(p, ts):
                proj_chains([(xk, wk_sb[p], kt[p][:, t * QC:(t + 1) * QC],
                              slice(t * QC, (t + 1) * QC)) for t in ts], f"k{p}")

            # before the pipeline: only K pair0 (first half) and Q chunk 0
            kproj(0, (0, 1))
            proj_chains([(xq, wq_sb[0], qt[0][0][:], slice(0, QC))], "q00")

            def emit_vproj(kc):
                """token-major V projection for key block kc: all 4 heads."""
                psv = psX.tile([128, 256], F32, tag="x", name=f"psv_{kc}")
                for d in range(ND):
                    nc.tensor.matmul(
                        psv[:], xv[d][:, kc * 128:(kc + 1) * 128],
                        wv4_sb[:, d * 256:(d + 1) * 256],
                        start=(d == 0), stop=(d == ND - 1))
                for h in range(4):
                    nc.vector.tensor_copy(v2[h][kc][:, 0:64],
                                          psv[:, h * 64:(h + 1) * 64])

            def emit_qproj(t):
                sl = slice(t * QC, (t + 1) * QC)
                proj_chains([(xq, wq_sb[p], qt[p][t][:], sl) for p in range(2)],
                            f"q_{t}")

            # ---- output projection ----
            ost_live = {}

            def emit_outproj_group(qc, o2t_pair, sub, oc, anchor):
                q0 = qc * QC
                if oc == 0:
                    ost_live[(qc, sub)] = osp.tile(
                        [128, D], F16, tag="os", name=f"os_{qc}_{sub}")
                ost = ost_live[(qc, sub)]
                pp = psX.tile([128, 512], F32, tag="x", name=f"pp_{qc}_{sub}_{oc}")
                for p in range(2):
                    mm = nc.tensor.matmul(
                        pp[:],
                        o2t_pair[p][:, sub * 128:(sub + 1) * 128],
                        wo_sb[p][:, oc * 512:(oc + 1) * 512],
                        start=(p == 0), stop=(p == 1))
                    if p == 0 and anchor is not None:
                        add_dep_helper(mm.ins, anchor.ins, sync=False,
                                       reason="interleave outproj after S")
                nc.vector.tensor_copy(ost[:, oc * 512:(oc + 1) * 512], pp[:])
                if oc == 1:
                    nc.sync.dma_start(
                        pout[q0 + sub * 128:q0 + (sub + 1) * 128, :],
                        ost[:])
                    del ost_live[(qc, sub)]

            # ---- 8-step attention pipeline ----
            steps = [(qc, p) for qc in range(NQ) for p in range(2)]
            prev = None           # (qc, p, po[2], at_list)
            o2t_all = {}          # (qc, p) -> o2t tile [128, QC] f16

            # insert schedule: remaining projections + V ride the early
            # steps' kb loops, ordered to match DMA arrival.
            inserts = {
                (0, 1): [lambda: kproj(1, (0, 1))],
                (0, 5): [lambda: proj_chains(
                    [(xq, wq_sb[1], qt[1][0][:], slice(0, QC))], "q10")],
                (0, 7): [lambda: kproj(0, (2,))],
                (0, 9): [lambda: emit_vproj(0)],
                (0, 10): [lambda: kproj(0, (3,))],
                (0, 11): [lambda: emit_vproj(1)],
                (0, 12): [lambda: kproj(1, (2,))],
                (0, 13): [lambda: emit_vproj(2)],
                (0, 14): [lambda: kproj(1, (3,))],
                (0, 15): [lambda: emit_vproj(3)],
                (1, 0): [lambda: emit_vproj(4)],
                (1, 1): [lambda: emit_vproj(5)],
                (1, 2): [lambda: emit_vproj(6)],
                (1, 3): [lambda: emit_vproj(7)],
                (1, 4): [lambda: emit_vproj(8)],
                (1, 5): [lambda: emit_vproj(9)],
                (1, 6): [lambda: emit_vproj(10)],
                (1, 7): [lambda: emit_vproj(11)],
                (1, 8): [lambda: emit_vproj(12)],
                (1, 10): [lambda: emit_qproj(1)],
                (1, 11): [lambda: emit_vproj(13)],
                (1, 12): [lambda: emit_vproj(14)],
                (1, 13): [lambda: emit_vproj(15)],
                (2, 3): [lambda: emit_qproj(2)],
                (2, 9): [lambda: emit_qproj(3)],
            }

            def emit_av(pr, kb):
                pqc, pp_, ppo, pat = pr
                for h in range(2):
                    nc.tensor.matmul(
                        ppo[h][:],
                        v2[2 * pp_ + h][kb][:],
                        pat[kb][:, h * QC:(h + 1) * QC],
                        start=(kb == 0), stop=(kb == NKB - 1))

            def emit_norm(pr):
                pqc, pp_, ppo, pat = pr
                o2t_t = o2tp.tile([128, QC], F16, tag=f"o2t{pp_}",
                                  name=f"o2t_{pqc}_{pp_}")
                for h in range(2):
                    d_sb = smallp.tile([1, QC], F32, tag=f"d{h}", name=f"d_{pqc}_{pp_}_{h}")
                    nc.vector.tensor_copy(d_sb[:], ppo[h][64:65, :])
                    r = smallp.tile([1, QC], F32, tag=f"r{h}", name=f"r_{pqc}_{pp_}_{h}")
                    nc.vector.reciprocal_approx_fast(r[:], d_sb[:])
                    rb = smallp.tile([64, QC], F32, tag=f"rb{h}", name=f"rb_{pqc}_{pp_}_{h}")
                    nc.gpsimd.partition_broadcast(rb[:], r[:])
                    nc.vector.tensor_mul(
                        o2t_t[h * 64:(h + 1) * 64, :],
                        ppo[h][0:64, :], rb[:])
                o2t_all[(pqc, pp_)] = o2t_t

            for s, (qc, p) in enumerate(steps):
                po = [psO.tile([65, QC], F32, tag=f"o{h}", name=f"po_{qc}_{p}_{h}")
                      for h in range(2)]
                at_list = []
                for kb in range(NKB):
                    k0 = kb * KB
                    ps = psS.tile([128, 2 * QC], F32, tag="s", name=f"s_{qc}_{p}_{kb}")
                    s_anchor = nc.tensor.matmul(
                        ps[:, 0:QC],
                        kt[p][0:64, k0:k0 + KB],
                        qt[p][qc][0:64, :],
                        start=True, stop=True, tile_position=(0, 0))
                    nc.tensor.matmul(
                        ps[:, QC:2 * QC],
                        kt[p][64:128, k0:k0 + KB],
                        qt[p][qc][64:128, :],
                        start=True, stop=True, tile_position=(64, 0))
                    at_t = atp.tile([128, 2 * QC], BF16, tag="at", name=f"at_{qc}_{p}_{kb}")
                    nc.scalar.activation(
                        at_t[:], ps[:], mybir.ActivationFunctionType.Exp)
                    at_list.append(at_t)

                    for fn in inserts.get((s, kb), ()):
                        fn()
                    if prev is not None:
                        emit_av(prev, kb)
                    # outproj of qc'=(s-3)//2 rides odd steps >= 3
                    if s >= 3 and s % 2 == 1 and kb % 2 == 1:
                        oqc = (s - 3) // 2
                        emit_outproj_group(
                            oqc,
                            [o2t_all[(oqc, 0)], o2t_all[(oqc, 1)]],
                            kb // 4, (kb // 2) % 2, s_anchor)
                if prev is not None:
                    emit_norm(prev)
                prev = (qc, p, po, at_list)

            # ---- tail: AV + norm of the last step, outproj of qc=3 ----
            for kb in range(NKB):
                emit_av(prev, kb)
            emit_norm(prev)
            for sub in range(4):
                for oc in range(2):
                    emit_outproj_group(
                        3, [o2t_all[(3, 0)], o2t_all[(3, 1)]], sub, oc, None)

    nc.compile()
    nc.m = get_hw_module(nc.m)
    return nc


def _pack_w(w_pair):
    # w_pair: [2, 1024, 64] -> [1024, 128] -> chunk-major [128, 8*128]
    w = np.concatenate([w_pair[0], w_pair[1]], axis=1)          # [1024, 128]
    return np.ascontiguousarray(
        w.reshape(ND, 128, 128).transpose(1, 0, 2).reshape(128, D))


def _pack_wv4(w_quad):
    # w_quad: [4, 1024, 64] -> [1024, 256] -> chunk-major [128, 8*256]
    w = np.concatenate([w_quad[h] for h in range(4)], axis=1)   # [1024, 256]
    return np.ascontiguousarray(
        w.reshape(ND, 128, 256).transpose(1, 0, 2).reshape(128, ND * 256))


def _pack_wo(wo_pair):
    # wo_pair: [2, 64, 1024] -> [128, 1024]
    return np.ascontiguousarray(np.concatenate([wo_pair[0], wo_pair[1]], axis=0))


def kernel(q, k, v, W_query, W_key, W_val, W_out, _trace=False):
    q = np.asarray(q, dtype=np.float32)
    k = np.asarray(k, dtype=np.float32)
    v = np.asarray(v, dtype=np.float32)
    W_query = np.asarray(W_query, dtype=np.float32)
    W_key = np.asarray(W_key, dtype=np.float32)
    W_val = np.asarray(W_val, dtype=np.float32)
    W_out = np.asarray(W_out, dtype=np.float32)

    if "nc" not in _CACHE:
        _CACHE["nc"] = _build()
    nc = _CACHE["nc"]

    norm = 1.0 / np.sqrt(E)
    xT = {}
    for b in range(2):
        xT[("q", b)] = np.ascontiguousarray(q[b].T).astype(np.float16)
        xT[("k", b)] = np.ascontiguousarray(k[b].T).astype(np.float16)
        xT[("v", b)] = np.ascontiguousarray(v[b].T).astype(np.float16)

    in_maps = []
    for c in range(N_CORES):
        b, g = c // 4, c % 4
        hs = [4 * g, 4 * g + 1, 4 * g + 2, 4 * g + 3]
        m = {
            "xqT": xT[("q", b)], "xkT": xT[("k", b)], "xvT": xT[("v", b)],
            "wv4": _pack_wv4(W_val[hs]).astype(np.float16),
        }
        for p in range(2):
            hp = hs[2 * p:2 * p + 2]
            m[f"wq{p}"] = _pack_w(W_query[hp] * norm).astype(np.float16)
            m[f"wk{p}"] = _pack_w(W_key[hp]).astype(np.float16)
            m[f"wo{p}"] = _pack_wo(W_out[hp]).astype(np.float16)
        in_maps.append(m)

    res = run_bass_kernel_spmd(nc, in_maps, list(range(N_CORES)),
                               trace=_trace)
    parts = [res.results[c]["pout"].astype(np.float32) for c in range(N_CORES)]
    out = np.stack([
        parts[0] + parts[1] + parts[2] + parts[3],
        parts[4] + parts[5] + parts[6] + parts[7],
    ])
    if _trace:
        _CACHE["last_result"] = res
    return out
